# revision 4
# baseline (speedup 1.0000x reference)
"""MoE feed-forward (top-2 of 8 experts) on 8 Trainium2 NeuronCores.

Strategy (expert-parallel, per the sharding hint):
  - The router (logits -> top-2 -> softmax gates) is the shard-assignment
    computation: it decides which tokens go to which core. It is 0.05% of
    the FLOPs and runs on the host as part of input sharding/dispatch.
  - Core e holds expert e's weights (E == n_cores == 8) and runs the
    dense MLP  gelu(x_e @ W1[e]) @ W2[e]  over the tokens routed to it,
    padded to a common capacity C so all cores run one SPMD program.
  - Everything on device is laid out transposed (tokens in the matmul
    free dim) so no on-device transposes are needed:
        hT[f, t] = sum_d W1[d, f] * xT[d, t]      (lhsT = W1 as stored)
        yT[d, t] = sum_f W2[f, d] * gelu(hT[f, t]) (lhsT = W2 as stored)
  - Matmuls run in bf16 (4x faster than fp32 on the PE) with fp32 PSUM
    accumulation; gelu in fp32 on the scalar engine; output fp32.
  - Host combine: out[tok] += gate * yT.T (scatter-add; each token
    appears at most once per expert so fancy-index add is exact).
"""

import numpy as np
import ml_dtypes

D = 1024
F = 4096
E = 8
TOPK = 2
P = 128

_BASS_CACHE: dict = {}


def _build_bass(
    C: int,
    d: int = D,
    f: int = F,
    tok_tile: int = 256,
    act: str = "Gelu",
    repeat: int = 1,
):
    import concourse.mybir as mybir
    import concourse.tile as tile
    from concourse import bacc

    act_fn = getattr(mybir.ActivationFunctionType, act)

    bf16 = mybir.dt.bfloat16
    f32 = mybir.dt.float32

    # Bacc (not plain Bass): its compile pipeline runs
    # generate_event_semaphores, which splits multi-sem waits into
    # InstEventSemaphore preludes — TRN2 instructions encode only 1 wait.
    nc = bacc.Bacc("TRN2", target_bir_lowering=False, debug=False, num_devices=E)
    xT = nc.declare_dram_parameter("xT", [d, C], bf16, isOutput=False)
    w1 = nc.declare_dram_parameter("w1", [d, f], bf16, isOutput=False)
    w2 = nc.declare_dram_parameter("w2", [f, d], bf16, isOutput=False)
    yT = nc.declare_dram_parameter("yT", [d, C], f32, isOutput=True)

    KD = d // P  # contraction tiles for mm1 / output d-tiles for mm2
    KF = f // P  # f-tiles for mm1 output / contraction tiles for mm2

    tts = []
    off = 0
    while off < C:
        tw = min(tok_tile, C - off)
        tts.append((off, tw))
        off += tw

    # HW-DGE DMA instructions can encode only ONE semaphore wait, and
    # once ~48 HW-DGE DMAs are in flight Tile adds a descriptor-ring
    # recycle wait to every later DMA. Stores inherently need a
    # data-ready wait, so the whole kernel must stay under ~48 DMAs:
    # batch the weight loads into one DMA each, one DMA per xt tile
    # (single-use slots, so no WAR/WAW waits), one store per token tile.
    with tile.TileContext(nc) as tc:
        with (
            tc.tile_pool(name="wpool", bufs=1) as wpool,
            tc.tile_pool(name="xpool", bufs=len(tts)) as xpool,
            tc.tile_pool(name="hpool", bufs=KF + 1) as hpool,
            tc.tile_pool(name="ypool", bufs=1) as ypool,
            tc.tile_pool(name="psum", bufs=8, space="PSUM") as psum_pool,
        ):
            w1_sb = wpool.tile([P, KD, f], bf16)
            nc.scalar.dma_start(w1_sb[:], w1.ap().rearrange("(ko p) f -> p ko f", p=P))
            w2_sb = wpool.tile([P, KF, d], bf16)
            nc.scalar.dma_start(w2_sb[:], w2.ap().rearrange("(ko p) f -> p ko f", p=P))

            xT_t = xT.ap().rearrange("(ko p) c -> p ko c", p=P)
            yT_t = yT.ap().rearrange("(ko p) c -> p ko c", p=P)
            xt_tiles = []
            for t0, tw in tts:
                xt = xpool.tile([P, KD, tok_tile], bf16, tag="xt", name="xt")[:, :, :tw]
                nc.scalar.dma_start(xt[:], xT_t[:, :, t0 : t0 + tw])
                xt_tiles.append(xt)

            for (t0, tw), xt in list(zip(tts, xt_tiles)) * repeat:
                h_tiles = []
                for ft in range(KF):
                    ps = psum_pool.tile([P, tok_tile], f32, tag="ps", name="ps")[:, :tw]
                    for k in range(KD):
                        nc.tensor.matmul(
                            ps[:],
                            w1_sb[:, k, ft * P : (ft + 1) * P],
                            xt[:, k, :],
                            start=(k == 0),
                            stop=(k == KD - 1),
                        )
                    h = hpool.tile([P, tok_tile], bf16, tag="h", name="h")[:, :tw]
                    nc.scalar.activation(h[:], ps[:], act_fn)
                    h_tiles.append(h)

                yt = ypool.tile([P, KD, tok_tile], f32, tag="yt", name="yt")[:, :, :tw]
                # wait-absorber: this DVE write takes on the slot's WAR
                # (previous store's DMA lane); the DVE copies below then
                # depend only on {PE, DVE} and the store only on {DVE} —
                # HW instruction encodings have very few sem-wait slots
                nc.vector.memset(yt[:], 0.0)
                for dt_ in range(KD):
                    ps2 = psum_pool.tile([P, tok_tile], f32, tag="ps", name="ps")[
                        :, :tw
                    ]
                    for ft in range(KF):
                        nc.tensor.matmul(
                            ps2[:],
                            w2_sb[:, ft, dt_ * P : (dt_ + 1) * P],
                            h_tiles[ft][:],
                            start=(ft == 0),
                            stop=(ft == KF - 1),
                        )
                    nc.vector.tensor_copy(yt[:, dt_, :], ps2[:])
                nc.sync.dma_start(yT_t[:, :, t0 : t0 + tw], yt[:])

    nc.compile()  # Bacc pipeline: reg alloc + wait splitting (1 wait/inst on TRN2)
    return nc


def _build_bass_v2(
    C: int,
    d: int = D,
    f: int = F,
    tok_tile: int = 512,
    act: str = "Gelu",
    repeat: int = 1,
):
    """v2: full-PSUM-bank matmuls (N=512) amortize per-MM issue/LDWEIGHTS
    overhead 2x vs 256; y staged and stored as bf16 (halves DVE copy and
    store-DMA traffic; adds ~1e-3 rel err, far under the 2e-2 budget)."""
    import concourse.mybir as mybir
    import concourse.tile as tile
    from concourse import bacc

    act_fn = getattr(mybir.ActivationFunctionType, act)

    bf16 = mybir.dt.bfloat16
    f32 = mybir.dt.float32

    nc = bacc.Bacc("TRN2", target_bir_lowering=False, debug=False, num_devices=E)
    xT = nc.declare_dram_parameter("xT", [d, C], bf16, isOutput=False)
    w1 = nc.declare_dram_parameter("w1", [d, f], bf16, isOutput=False)
    w2 = nc.declare_dram_parameter("w2", [f, d], bf16, isOutput=False)
    yT = nc.declare_dram_parameter("yT", [d, C], bf16, isOutput=True)

    KD = d // P
    KF = f // P

    tts = []
    off = 0
    while off < C:
        tw = min(tok_tile, C - off)
        tts.append((off, tw))
        off += tw

    # hpool NEEDS >= KF bufs: mm2's last d-group reads every h tile, so all
    # KF h tiles of one iteration are live at once (KF-1 deadlocks).
    # xt tiles are sized per width (tag per tw) so the 128-wide tail doesn't
    # pay for a full 512-wide slot — SBUF is within ~4KB of full here.
    from collections import Counter

    tw_counts = Counter(tw for _, tw in tts)
    with tile.TileContext(nc) as tc:
        with (
            tc.tile_pool(name="wpool", bufs=1) as wpool,
            tc.tile_pool(name="xpool", bufs=1) as xpool,
            tc.tile_pool(name="hpool", bufs=KF) as hpool,
            tc.tile_pool(name="ypool", bufs=1) as ypool,
            tc.tile_pool(name="psum", bufs=8, space="PSUM") as psum_pool,
        ):
            w1_sb = wpool.tile([P, KD, f], bf16)
            nc.scalar.dma_start(w1_sb[:], w1.ap().rearrange("(ko p) f -> p ko f", p=P))
            w2_sb = wpool.tile([P, KF, d], bf16)
            nc.scalar.dma_start(w2_sb[:], w2.ap().rearrange("(ko p) f -> p ko f", p=P))

            xT_t = xT.ap().rearrange("(ko p) c -> p ko c", p=P)
            yT_t = yT.ap().rearrange("(ko p) c -> p ko c", p=P)
            xt_tiles = []
            for t0, tw in tts:
                xt = xpool.tile(
                    [P, KD, tw], bf16, tag=f"xt{tw}", bufs=tw_counts[tw], name="xt"
                )
                nc.scalar.dma_start(xt[:], xT_t[:, :, t0 : t0 + tw])
                xt_tiles.append(xt)

            for (t0, tw), xt in list(zip(tts, xt_tiles)) * repeat:
                h_tiles = []
                for ft in range(KF):
                    ps = psum_pool.tile([P, tok_tile], f32, tag="ps", name="ps")[:, :tw]
                    for k in range(KD):
                        nc.tensor.matmul(
                            ps[:],
                            w1_sb[:, k, ft * P : (ft + 1) * P],
                            xt[:, k, :],
                            start=(k == 0),
                            stop=(k == KD - 1),
                        )
                    h = hpool.tile([P, tok_tile], bf16, tag="h", name="h")[:, :tw]
                    nc.scalar.activation(h[:], ps[:], act_fn)
                    h_tiles.append(h)

                yt = ypool.tile([P, KD, tok_tile], bf16, tag="yt", name="yt")[:, :, :tw]
                nc.vector.memset(yt[:], 0.0)
                for dt_ in range(KD):
                    ps2 = psum_pool.tile([P, tok_tile], f32, tag="ps", name="ps")[
                        :, :tw
                    ]
                    for ft in range(KF):
                        nc.tensor.matmul(
                            ps2[:],
                            w2_sb[:, ft, dt_ * P : (dt_ + 1) * P],
                            h_tiles[ft][:],
                            start=(ft == 0),
                            stop=(ft == KF - 1),
                        )
                    nc.vector.tensor_copy(yt[:, dt_, :], ps2[:])
                nc.sync.dma_start(yT_t[:, :, t0 : t0 + tw], yt[:])

    nc.compile()
    return nc


def _route(xf: np.ndarray, Wr: np.ndarray):
    """Top-2 routing on the host (fp64 logits for a stable ranking)."""
    logits = xf.astype(np.float64) @ Wr.astype(np.float64).T  # [N, E]
    order = np.argsort(-logits, axis=1)[:, :TOPK]  # [N, 2] expert ids, desc
    top_vals = np.take_along_axis(logits, order, axis=1).astype(np.float32)
    m = top_vals.max(axis=1, keepdims=True)
    ex = np.exp(top_vals - m)
    gates2 = (ex / ex.sum(axis=1, keepdims=True)).astype(np.float32)  # [N, 2]
    return order, gates2


def _run(inputs, trace: bool = False):
    x = np.asarray(inputs["x"], dtype=np.float32)
    Wr = np.asarray(inputs["Wr"], dtype=np.float32)
    W1 = np.asarray(inputs["W1"], dtype=np.float32)
    W2 = np.asarray(inputs["W2"], dtype=np.float32)
    B, T, d = x.shape
    N = B * T
    xf = np.ascontiguousarray(x.reshape(N, d))

    order, gates2 = _route(xf, Wr)

    counts = np.bincount(order.ravel(), minlength=E)
    C = int(-(-max(int(counts.max()), P) // P) * P)  # ceil to multiple of 128

    idx_list, gate_list = [], []
    for e in range(E):
        tok, slot = np.where(order == e)
        idx_list.append(tok)
        gate_list.append(gates2[tok, slot])

    xf_bf = xf.astype(ml_dtypes.bfloat16)
    in_maps = []
    for e in range(E):
        xTe = np.zeros((d, C), dtype=ml_dtypes.bfloat16)
        tok = idx_list[e]
        xTe[:, : len(tok)] = xf_bf[tok].T
        in_maps.append(
            {
                "xT": xTe,
                "w1": np.ascontiguousarray(W1[e]).astype(ml_dtypes.bfloat16),
                "w2": np.ascontiguousarray(W2[e]).astype(ml_dtypes.bfloat16),
            }
        )

    key = (C, d)
    if key not in _BASS_CACHE:
        _BASS_CACHE[key] = _build_bass(C, d=d, f=W1.shape[2])
    nc = _BASS_CACHE[key]

    from concourse.bass_utils import run_bass_kernel_spmd

    res = run_bass_kernel_spmd(nc, in_maps, core_ids=list(range(E)), trace=trace)

    out = np.zeros((N, d), dtype=np.float32)
    for e in range(E):
        tok = idx_list[e]
        yTe = np.asarray(res.results[e]["yT"])  # [d, C] fp32
        out[tok] += gate_list[e][:, None] * yTe[:, : len(tok)].T
    return out.reshape(B, T, d), res


def kernel(**inputs) -> np.ndarray:
    out, _ = _run(inputs, trace=False)
    return out



# revision 5
# speedup vs baseline: 1485.8755x; 1485.8755x over previous
"""MoE feed-forward (top-2 of 8 experts) on 8 Trainium2 NeuronCores.

Strategy (expert-parallel, per the sharding hint):
  - The router (logits -> top-2 -> softmax gates) is the shard-assignment
    computation: it decides which tokens go to which core. It is 0.05% of
    the FLOPs and runs on the host as part of input sharding/dispatch.
  - Core e holds expert e's weights (E == n_cores == 8) and runs the
    dense MLP  gelu(x_e @ W1[e]) @ W2[e]  over the tokens routed to it,
    padded to a common capacity C so all cores run one SPMD program.
  - Everything on device is laid out transposed (tokens in the matmul
    free dim) so no on-device transposes are needed:
        hT[f, t] = sum_d W1[d, f] * xT[d, t]      (lhsT = W1 as stored)
        yT[d, t] = sum_f W2[f, d] * gelu(hT[f, t]) (lhsT = W2 as stored)
  - Matmuls run in bf16 (4x faster than fp32 on the PE) with fp32 PSUM
    accumulation; gelu in fp32 on the scalar engine; output fp32.
  - Host combine: out[tok] += gate * yT.T (scatter-add; each token
    appears at most once per expert so fancy-index add is exact).
"""

import numpy as np
import ml_dtypes

D = 1024
F = 4096
E = 8
TOPK = 2
P = 128

_BASS_CACHE: dict = {}


def _build_bass(
    C: int,
    d: int = D,
    f: int = F,
    tok_tile: int = 256,
    act: str = "Gelu",
    repeat: int = 1,
):
    import concourse.mybir as mybir
    import concourse.tile as tile
    from concourse import bacc

    act_fn = getattr(mybir.ActivationFunctionType, act)

    bf16 = mybir.dt.bfloat16
    f32 = mybir.dt.float32

    # Bacc (not plain Bass): its compile pipeline runs
    # generate_event_semaphores, which splits multi-sem waits into
    # InstEventSemaphore preludes — TRN2 instructions encode only 1 wait.
    nc = bacc.Bacc("TRN2", target_bir_lowering=False, debug=False, num_devices=E)
    xT = nc.declare_dram_parameter("xT", [d, C], bf16, isOutput=False)
    w1 = nc.declare_dram_parameter("w1", [d, f], bf16, isOutput=False)
    w2 = nc.declare_dram_parameter("w2", [f, d], bf16, isOutput=False)
    yT = nc.declare_dram_parameter("yT", [d, C], f32, isOutput=True)

    KD = d // P  # contraction tiles for mm1 / output d-tiles for mm2
    KF = f // P  # f-tiles for mm1 output / contraction tiles for mm2

    tts = []
    off = 0
    while off < C:
        tw = min(tok_tile, C - off)
        tts.append((off, tw))
        off += tw

    # HW-DGE DMA instructions can encode only ONE semaphore wait, and
    # once ~48 HW-DGE DMAs are in flight Tile adds a descriptor-ring
    # recycle wait to every later DMA. Stores inherently need a
    # data-ready wait, so the whole kernel must stay under ~48 DMAs:
    # batch the weight loads into one DMA each, one DMA per xt tile
    # (single-use slots, so no WAR/WAW waits), one store per token tile.
    with tile.TileContext(nc) as tc:
        with (
            tc.tile_pool(name="wpool", bufs=1) as wpool,
            tc.tile_pool(name="xpool", bufs=len(tts)) as xpool,
            tc.tile_pool(name="hpool", bufs=KF + 1) as hpool,
            tc.tile_pool(name="ypool", bufs=1) as ypool,
            tc.tile_pool(name="psum", bufs=8, space="PSUM") as psum_pool,
        ):
            w1_sb = wpool.tile([P, KD, f], bf16)
            nc.scalar.dma_start(w1_sb[:], w1.ap().rearrange("(ko p) f -> p ko f", p=P))
            w2_sb = wpool.tile([P, KF, d], bf16)
            nc.scalar.dma_start(w2_sb[:], w2.ap().rearrange("(ko p) f -> p ko f", p=P))

            xT_t = xT.ap().rearrange("(ko p) c -> p ko c", p=P)
            yT_t = yT.ap().rearrange("(ko p) c -> p ko c", p=P)
            xt_tiles = []
            for t0, tw in tts:
                xt = xpool.tile([P, KD, tok_tile], bf16, tag="xt", name="xt")[:, :, :tw]
                nc.scalar.dma_start(xt[:], xT_t[:, :, t0 : t0 + tw])
                xt_tiles.append(xt)

            for (t0, tw), xt in list(zip(tts, xt_tiles)) * repeat:
                h_tiles = []
                for ft in range(KF):
                    ps = psum_pool.tile([P, tok_tile], f32, tag="ps", name="ps")[:, :tw]
                    for k in range(KD):
                        nc.tensor.matmul(
                            ps[:],
                            w1_sb[:, k, ft * P : (ft + 1) * P],
                            xt[:, k, :],
                            start=(k == 0),
                            stop=(k == KD - 1),
                        )
                    h = hpool.tile([P, tok_tile], bf16, tag="h", name="h")[:, :tw]
                    nc.scalar.activation(h[:], ps[:], act_fn)
                    h_tiles.append(h)

                yt = ypool.tile([P, KD, tok_tile], f32, tag="yt", name="yt")[:, :, :tw]
                # wait-absorber: this DVE write takes on the slot's WAR
                # (previous store's DMA lane); the DVE copies below then
                # depend only on {PE, DVE} and the store only on {DVE} —
                # HW instruction encodings have very few sem-wait slots
                nc.vector.memset(yt[:], 0.0)
                for dt_ in range(KD):
                    ps2 = psum_pool.tile([P, tok_tile], f32, tag="ps", name="ps")[
                        :, :tw
                    ]
                    for ft in range(KF):
                        nc.tensor.matmul(
                            ps2[:],
                            w2_sb[:, ft, dt_ * P : (dt_ + 1) * P],
                            h_tiles[ft][:],
                            start=(ft == 0),
                            stop=(ft == KF - 1),
                        )
                    nc.vector.tensor_copy(yt[:, dt_, :], ps2[:])
                nc.sync.dma_start(yT_t[:, :, t0 : t0 + tw], yt[:])

    nc.compile()  # Bacc pipeline: reg alloc + wait splitting (1 wait/inst on TRN2)
    return nc


def _build_bass_v2(
    C: int,
    d: int = D,
    f: int = F,
    tok_tile: int = 512,
    act: str = "Gelu",
    repeat: int = 1,
):
    """v2: full-PSUM-bank matmuls (N=512) amortize per-MM issue/LDWEIGHTS
    overhead 2x vs 256; y staged and stored as bf16 (halves DVE copy and
    store-DMA traffic; adds ~1e-3 rel err, far under the 2e-2 budget)."""
    import concourse.mybir as mybir
    import concourse.tile as tile
    from concourse import bacc

    act_fn = getattr(mybir.ActivationFunctionType, act)

    bf16 = mybir.dt.bfloat16
    f32 = mybir.dt.float32

    nc = bacc.Bacc("TRN2", target_bir_lowering=False, debug=False, num_devices=E)
    xT = nc.declare_dram_parameter("xT", [d, C], bf16, isOutput=False)
    w1 = nc.declare_dram_parameter("w1", [d, f], bf16, isOutput=False)
    w2 = nc.declare_dram_parameter("w2", [f, d], bf16, isOutput=False)
    yT = nc.declare_dram_parameter("yT", [d, C], bf16, isOutput=True)

    KD = d // P
    KF = f // P

    tts = []
    off = 0
    while off < C:
        tw = min(tok_tile, C - off)
        tts.append((off, tw))
        off += tw

    # hpool NEEDS >= KF bufs: mm2's last d-group reads every h tile, so all
    # KF h tiles of one iteration are live at once (KF-1 deadlocks).
    # xt tiles are sized per width (tag per tw) so the 128-wide tail doesn't
    # pay for a full 512-wide slot — SBUF is within ~4KB of full here.
    from collections import Counter

    tw_counts = Counter(tw for _, tw in tts)
    with tile.TileContext(nc) as tc:
        with (
            tc.tile_pool(name="wpool", bufs=1) as wpool,
            tc.tile_pool(name="xpool", bufs=1) as xpool,
            tc.tile_pool(name="hpool", bufs=KF) as hpool,
            tc.tile_pool(name="ypool", bufs=1) as ypool,
            tc.tile_pool(name="psum", bufs=8, space="PSUM") as psum_pool,
        ):
            w1_sb = wpool.tile([P, KD, f], bf16)
            nc.scalar.dma_start(w1_sb[:], w1.ap().rearrange("(ko p) f -> p ko f", p=P))
            w2_sb = wpool.tile([P, KF, d], bf16)
            nc.scalar.dma_start(w2_sb[:], w2.ap().rearrange("(ko p) f -> p ko f", p=P))

            xT_t = xT.ap().rearrange("(ko p) c -> p ko c", p=P)
            yT_t = yT.ap().rearrange("(ko p) c -> p ko c", p=P)
            xt_tiles = []
            for t0, tw in tts:
                xt = xpool.tile(
                    [P, KD, tw], bf16, tag=f"xt{tw}", bufs=tw_counts[tw], name="xt"
                )
                nc.scalar.dma_start(xt[:], xT_t[:, :, t0 : t0 + tw])
                xt_tiles.append(xt)

            for (t0, tw), xt in list(zip(tts, xt_tiles)) * repeat:
                h_tiles = []
                for ft in range(KF):
                    ps = psum_pool.tile([P, tok_tile], f32, tag="ps", name="ps")[:, :tw]
                    for k in range(KD):
                        nc.tensor.matmul(
                            ps[:],
                            w1_sb[:, k, ft * P : (ft + 1) * P],
                            xt[:, k, :],
                            start=(k == 0),
                            stop=(k == KD - 1),
                        )
                    h = hpool.tile([P, tok_tile], bf16, tag="h", name="h")[:, :tw]
                    nc.scalar.activation(h[:], ps[:], act_fn)
                    h_tiles.append(h)

                yt = ypool.tile([P, KD, tok_tile], bf16, tag="yt", name="yt")[:, :, :tw]
                nc.vector.memset(yt[:], 0.0)
                for dt_ in range(KD):
                    ps2 = psum_pool.tile([P, tok_tile], f32, tag="ps", name="ps")[
                        :, :tw
                    ]
                    for ft in range(KF):
                        nc.tensor.matmul(
                            ps2[:],
                            w2_sb[:, ft, dt_ * P : (dt_ + 1) * P],
                            h_tiles[ft][:],
                            start=(ft == 0),
                            stop=(ft == KF - 1),
                        )
                    nc.vector.tensor_copy(yt[:, dt_, :], ps2[:])
                nc.sync.dma_start(yT_t[:, :, t0 : t0 + tw], yt[:])

    nc.compile()
    return nc


def _route(xf: np.ndarray, Wr: np.ndarray):
    """Top-2 routing on the host (fp64 logits for a stable ranking)."""
    logits = xf.astype(np.float64) @ Wr.astype(np.float64).T  # [N, E]
    order = np.argsort(-logits, axis=1)[:, :TOPK]  # [N, 2] expert ids, desc
    top_vals = np.take_along_axis(logits, order, axis=1).astype(np.float32)
    m = top_vals.max(axis=1, keepdims=True)
    ex = np.exp(top_vals - m)
    gates2 = (ex / ex.sum(axis=1, keepdims=True)).astype(np.float32)  # [N, 2]
    return order, gates2


def _run(inputs, trace: bool = False):
    x = np.asarray(inputs["x"], dtype=np.float32)
    Wr = np.asarray(inputs["Wr"], dtype=np.float32)
    W1 = np.asarray(inputs["W1"], dtype=np.float32)
    W2 = np.asarray(inputs["W2"], dtype=np.float32)
    B, T, d = x.shape
    N = B * T
    xf = np.ascontiguousarray(x.reshape(N, d))

    order, gates2 = _route(xf, Wr)

    counts = np.bincount(order.ravel(), minlength=E)
    C = int(-(-max(int(counts.max()), P) // P) * P)  # ceil to multiple of 128

    idx_list, gate_list = [], []
    for e in range(E):
        tok, slot = np.where(order == e)
        idx_list.append(tok)
        gate_list.append(gates2[tok, slot])

    xf_bf = xf.astype(ml_dtypes.bfloat16)
    in_maps = []
    for e in range(E):
        xTe = np.zeros((d, C), dtype=ml_dtypes.bfloat16)
        tok = idx_list[e]
        xTe[:, : len(tok)] = xf_bf[tok].T
        in_maps.append(
            {
                "xT": xTe,
                "w1": np.ascontiguousarray(W1[e]).astype(ml_dtypes.bfloat16),
                "w2": np.ascontiguousarray(W2[e]).astype(ml_dtypes.bfloat16),
            }
        )

    key = (C, d)
    if key not in _BASS_CACHE:
        _BASS_CACHE[key] = _build_bass_v2(C, d=d, f=W1.shape[2])
    nc = _BASS_CACHE[key]

    from concourse.bass_utils import run_bass_kernel_spmd

    res = run_bass_kernel_spmd(nc, in_maps, core_ids=list(range(E)), trace=trace)

    out = np.zeros((N, d), dtype=np.float32)
    for e in range(E):
        tok = idx_list[e]
        yTe = np.asarray(res.results[e]["yT"]).astype(np.float32)  # [d, C]
        out[tok] += gate_list[e][:, None] * yTe[:, : len(tok)].T
    return out.reshape(B, T, d), res


def kernel(**inputs) -> np.ndarray:
    out, _ = _run(inputs, trace=False)
    return out

